# revision 1
# baseline (speedup 1.0000x reference)
"""Trainium2 Bass kernel for nn_CoAttentionLayer2 (dense_transformer).

Sharding: pure data parallel — batch B=8 mapped 1:1 onto 8 NeuronCores.
Each core runs the full co-attention layer for one batch element; no
collectives. Weights are replicated.

Schedule (v3): organized around a continuous ACT exp stream — exp of all
8.4M logits on the scalar engine is the hard floor (~70us):

  prologue: x tiles DMA on the sync queue, weights on the gpsimd queue.
    LN per token tile: DVE bn_stats/bn_aggr/reciprocal, gpsimd -mu*rstd,
    ACT sqrt + xhat affine (per-partition scale/bias), PE transpose
    (fp16), PSUM->SBUF copyback load-balanced between DVE and ACT.
    Emission order feeds pair-0 attention ASAP: ln(kv0-3), K(0,0),
    ln(q0-3), Q(0,0) -> first exp ~27us; remaining LN/projections
    overlap the early exp stream.
  attention: 64 steps of (dots pair -> exp -> attn@v pair); head pairs
    share the PE via disjoint row groups.  Remaining projections trickle
    in as one-matmul fillers on a dedicated 1-bank PSUM ring; their
    copybacks run on DVE (ACT is 100% exp).  attn@v accumulates per
    (head, query-chunk) into 1-bank PSUM tiles; softmax row sums ride an
    augmented ones-column in V.  Pending attn@v work drains early near
    each chunk boundary so the po buffer (bufs=1) frees before the next
    chunk's first attn@v needs it.  Output projection for the first
    token half runs as late fillers once all pairs' first-chunk
    normalize has been emitted.
  tail: output projection for the second token half; results DMA
    straight from PSUM to DRAM (no SBUF bounce).

PSUM budget (8 banks): dots ring 2x[128,1024] = 4, filler ring
2x[128,512] = 2, attn@v po0/po1 1 bank each = 2.
"""

import collections

import numpy as np

import concourse.bass as bass
import concourse.mybir as mybir
import concourse.tile as tile
from concourse import bacc
from concourse.bass_utils import run_bass_kernel_spmd
from concourse.masks import make_identity

P = 128
B = 8
N = 1024  # tokens (queries == keys)
D = 512  # model dim
HEADS = 8
DH = 64
INNER = 512
SCALE = DH**-0.5
EPS = 1e-5
F32 = mybir.dt.float32
F32R = mybir.dt.float32r
F16 = mybir.dt.float16

KO = D // P  # 4 contraction tiles
JT = INNER // P  # 4 output-feature tiles (== head pairs)
TT = N // P  # 8 token tiles
IC = 2  # query/token chunks of 512
NQC = N // IC  # 512
LAG = 4  # attn@v trails dots/exp by this many steps (mid-phase)
EX_BUFS = LAG + 4
# pending-av threshold per phase step: drains the old chunk's attn@v
# early so its po bank frees ~2 steps before the next chunk's first
# attn@v (po bufs=1), without bursting more than 2 av pairs per step
AV_THRESH = [3, 2, 3, 4, 4, 4, 3, 2]


def _build_nc():
    nc = bacc.Bacc(
        "TRN2",
        target_bir_lowering=False,
        debug=False,
        num_devices=B,
    )

    xq_d = nc.declare_dram_parameter("xq", [N, D], F32, isOutput=False)
    xkv_d = nc.declare_dram_parameter("xkv", [N, D], F32, isOutput=False)
    wq_d = nc.declare_dram_parameter("wq", [D, INNER], F16, isOutput=False)
    wk_d = nc.declare_dram_parameter("wk", [D, INNER], F16, isOutput=False)
    wv_d = nc.declare_dram_parameter("wv", [D, INNER], F16, isOutput=False)
    wo_d = nc.declare_dram_parameter("wo", [INNER, D], F32R, isOutput=False)
    bq_d = nc.declare_dram_parameter("bq", [INNER], F32, isOutput=False)
    bk_d = nc.declare_dram_parameter("bk", [INNER], F32, isOutput=False)
    bv_d = nc.declare_dram_parameter("bv", [INNER], F32R, isOutput=False)
    out_d = nc.declare_dram_parameter("out", [N, D], F32, isOutput=True)

    with tile.TileContext(nc) as tc:
        with (
            tc.tile_pool(name="singles", bufs=1) as singles,
            tc.tile_pool(name="big", bufs=1) as big,
            tc.tile_pool(name="work", bufs=3) as work,
            tc.tile_pool(name="ps", bufs=2, space="PSUM") as ps,
        ):
            eps_sb = singles.tile([P, 1], F32)
            nc.vector.memset(eps_sb, EPS)

            ident = singles.tile([P, P], F32)
            make_identity(nc, ident)

            # ---- weights on the gpsimd-triggered DMA queue ----
            bq_sb = singles.tile([P, JT], F32)
            bk_sb = singles.tile([P, JT], F32)
            nc.gpsimd.dma_start(out=bq_sb[:], in_=bq_d.rearrange("(t p) -> p t", p=P))
            nc.gpsimd.dma_start(out=bk_sb[:], in_=bk_d.rearrange("(t p) -> p t", p=P))
            bv_row = singles.tile([1, INNER], F32R)
            bv_ap = bv_d.ap()
            nc.gpsimd.dma_start(
                out=bv_row[:],
                in_=bass.AP(tensor=bv_ap.tensor, offset=bv_ap.offset, ap=[[0, 1], [1, INNER]]),
            )
            ones_row_f32 = singles.tile([1, NQC], F32)
            nc.vector.memset(ones_row_f32, 1.0)
            ones_row = ones_row_f32.bitcast(F32R)

            wk_sb = singles.tile([P, KO, INNER], F16)
            wq_sb = singles.tile([P, KO, INNER], F16)
            wv_sb = singles.tile([P, KO, INNER], F16)
            wo_sb = singles.tile([P, KO, D], F32R)
            nc.gpsimd.dma_start(out=wk_sb[:], in_=wk_d.rearrange("(ko p) j -> p ko j", p=P))
            nc.gpsimd.dma_start(out=wq_sb[:], in_=wq_d.rearrange("(ko p) j -> p ko j", p=P))
            nc.gpsimd.dma_start(out=wv_sb[:], in_=wv_d.rearrange("(ko p) j -> p ko j", p=P))
            nc.gpsimd.dma_start(out=wo_sb[:], in_=wo_d.rearrange("(co p) j -> p co j", p=P))

            # ---- persistent activations ----
            xhatT_q = big.tile([P, KO, N], F16)  # [d%128, d//128, token]
            xhatT_kv = big.tile([P, KO, N], F16)
            QT = big.tile([P, JT, N], F16)  # [j%128, j//128, token]
            KT = big.tile([P, JT, N], F16)
            Vg = big.tile([P, TT, HEADS, DH + 1], F16)  # [key%128, keytile, h, dh|1]
            outT = big.tile([P, KO, N], F32R)  # [c%128, c//128, token]

            ones_sb = singles.tile([P, 1], F32)
            nc.vector.memset(ones_sb, 1.0)
            nc.vector.tensor_copy(
                out=Vg[:, :, :, DH : DH + 1],
                in_=ones_sb[:, None, None, :].to_broadcast((P, TT, HEADS, 1)),
            )

            # prologue DVE/ACT load balance (ns emitted so far)
            load = {"dve": 0.0, "act": 0.0}

            def lighter():
                return "dve" if load["dve"] <= load["act"] else "act"

            # ---- LayerNorm + transpose ----
            def ln_transpose(x_d, xhatT, tt):
                xt = work.tile([P, D], F32, tag="ln_in", bufs=6)
                nc.sync.dma_start(out=xt[:], in_=x_d[tt * P : (tt + 1) * P, :])
                stats = work.tile([P, 6], F32, tag="ln_stats")
                nc.vector.bn_stats(out=stats[:], in_=xt[:])
                mv = work.tile([P, 2], F32, tag="ln_mv")
                nc.vector.bn_aggr(out=mv[:], in_=stats[:])
                std = work.tile([P, 1], F32, tag="ln_std")
                nc.scalar.activation(
                    out=std[:],
                    in_=mv[:, 1:2],
                    func=mybir.ActivationFunctionType.Sqrt,
                    bias=eps_sb[:],
                    scale=1.0,
                )
                rstd = work.tile([P, 1], F32, tag="ln_rstd")
                nc.vector.reciprocal(out=rstd[:], in_=std[:])
                nmr = work.tile([P, 1], F32, tag="ln_nmr")
                # nmr = -(mu * rstd)
                nc.gpsimd.tensor_scalar(
                    out=nmr[:],
                    in0=mv[:, 0:1],
                    scalar1=rstd[:],
                    scalar2=-1.0,
                    op0=mybir.AluOpType.mult,
                    op1=mybir.AluOpType.mult,
                )
                xhat = work.tile([P, D], F32, tag="xhat")
                nc.scalar.activation(
                    out=xhat[:],
                    in_=xt[:],
                    func=mybir.ActivationFunctionType.Identity,
                    bias=nmr[:],
                    scale=rstd[:],
                )
                load["dve"] += 1050
                load["act"] += 1150
                pt = ps.tile([P, D], F32, tag="big", name="pt")
                for db in range(KO):
                    nc.tensor.transpose(
                        pt[:, db * P : (db + 1) * P], xhat[:, db * P : (db + 1) * P], ident[:]
                    )
                dst = xhatT[:, :, tt * P : (tt + 1) * P]
                src = pt[:].rearrange("p (ko t) -> p ko t", t=P)
                if lighter() == "dve":
                    nc.vector.tensor_copy(out=dst, in_=src)
                    load["dve"] += 450
                else:
                    nc.scalar.copy(out=dst, in_=src)
                    load["act"] += 700

            # ---- projection chunks (each = KO matmuls [+1] + 1 copyback) ----
            def qk_units(w_sb, b_sb, src, dstT, jt, ic, cb_eng=None):
                box = {}

                def mk(ko):
                    def f():
                        if ko == 0:
                            box["pm"] = ps.tile([P, NQC], F32, tag="pm", name="pm")
                        nc.tensor.matmul(
                            box["pm"][:],
                            w_sb[:, ko, jt * P : (jt + 1) * P],
                            src[:, ko, ic * NQC : (ic + 1) * NQC],
                            start=(ko == 0),
                            stop=(ko == KO - 1),
                        )

                    return f

                def copy():
                    eng = cb_eng or lighter()
                    dst = dstT[:, jt, ic * NQC : (ic + 1) * NQC]
                    if eng == "dve":
                        nc.vector.tensor_scalar(
                            out=dst,
                            in0=box["pm"][:],
                            scalar1=b_sb[:, jt : jt + 1],
                            scalar2=None,
                            op0=mybir.AluOpType.add,
                        )
                        load["dve"] += 800
                    else:
                        nc.scalar.activation(
                            out=dst,
                            in_=box["pm"][:],
                            func=mybir.ActivationFunctionType.Identity,
                            bias=b_sb[:, jt : jt + 1],
                            scale=1.0,
                        )
                        load["act"] += 850

                return [mk(ko) for ko in range(KO)] + [copy]

            def v_units(tt):
                box = {}

                def mk(ko):
                    def f():
                        if ko == 0:
                            box["pm"] = ps.tile([P, NQC], F32, tag="pm", name="pmv")
                        nc.tensor.matmul(
                            box["pm"][:],
                            xhatT_kv[:, ko, tt * P : (tt + 1) * P],
                            wv_sb[:, ko, :],
                            start=(ko == 0),
                            stop=False,
                        )

                    return f

                def bias_mm():
                    # pm += ones ⊗ bv  (rank-1 bias add on the PE)
                    nc.tensor.matmul(
                        box["pm"][:],
                        ones_row[0:1, 0:P],
                        bv_row[0:1, :],
                        start=False,
                        stop=True,
                    )

                def copy():
                    dst = Vg[:, tt, :, 0:DH]
                    src = box["pm"][:].rearrange("p (h d) -> p h d", d=DH)
                    if lighter() == "dve":
                        nc.vector.tensor_copy(out=dst, in_=src)
                        load["dve"] += 750
                    else:
                        nc.scalar.copy(out=dst, in_=src)
                        load["act"] += 750

                return [mk(ko) for ko in range(KO)] + [bias_mm, copy]

            def emit_chunk(units):
                for u in units:
                    u()

            # ---- attention ----
            pending_av = collections.deque()

            def do_av(pair, ic, kt, ex, po):
                for hh in range(2):
                    h = 2 * pair + hh
                    nc.tensor.matmul(
                        po[hh][:, :],
                        Vg[:, kt, h, :],
                        ex[:, hh * NQC : (hh + 1) * NQC],
                        start=(kt == 0),
                        stop=(kt == TT - 1),
                    )
                if kt == TT - 1:
                    normalize(pair, ic, po)

            def normalize(pair, ic, po):
                for hh in range(2):
                    hb = hh * DH
                    rtmp = work.tile([1, 2 * NQC], F32, tag="rectmp", bufs=4)
                    rs, rec = rtmp[:, 0:NQC], rtmp[:, NQC : 2 * NQC]
                    nc.vector.tensor_copy(out=rs, in_=po[hh][DH : DH + 1, :])
                    nc.vector.reciprocal_approx_fast(out=rec, in_=rs)
                    recB = work.tile([DH, NQC], F32, tag="recB", bufs=4)
                    nc.gpsimd.partition_broadcast(recB[:], rec)
                    nc.vector.tensor_tensor(
                        out=outT[hb : hb + DH, pair, ic * NQC : (ic + 1) * NQC],
                        in0=po[hh][0:DH, :],
                        in1=recB[:],
                        op=mybir.AluOpType.mult,
                    )

            def emit_step(pair, ic, kt, po):
                pd = ps.tile([P, N], F32, tag="big", name="pd")
                for hh in range(2):
                    nc.tensor.matmul(
                        pd[:, hh * NQC : (hh + 1) * NQC],
                        KT[hh * DH : (hh + 1) * DH, pair, kt * P : (kt + 1) * P],
                        QT[hh * DH : (hh + 1) * DH, pair, ic * NQC : (ic + 1) * NQC],
                        start=True,
                        stop=True,
                        tile_position=(hh * DH, 0),
                    )
                ex = work.tile([P, N], F16, tag="expT", bufs=EX_BUFS)
                nc.scalar.activation(
                    out=ex[:],
                    in_=pd[:],
                    func=mybir.ActivationFunctionType.Exp,
                    scale=SCALE,
                )
                pending_av.append((pair, ic, kt, ex, po))

            # ---- output projection ----
            def o_units(tt, cb_eng="act"):
                box = {}

                def mk(co):
                    def f():
                        if co == 0:
                            box["pm"] = ps.tile([P, NQC], F32, tag="pm", name="pmo")
                        nc.tensor.matmul(
                            box["pm"][:],
                            outT[:, co, tt * P : (tt + 1) * P],
                            wo_sb[:, co, :],
                            start=(co == 0),
                            stop=(co == KO - 1),
                        )

                    return f

                def copy_dma():
                    ot = work.tile([P, D], F32, tag="out", bufs=3)
                    if cb_eng == "dve":
                        nc.vector.tensor_copy(out=ot[:], in_=box["pm"][:])
                    else:
                        nc.scalar.copy(out=ot[:], in_=box["pm"][:])
                    nc.sync.dma_start(out=out_d[tt * P : (tt + 1) * P, :], in_=ot[:])

                return [mk(co) for co in range(KO)] + [copy_dma]

            # ================= emission =================
            # prologue: pair-0 dependencies first
            for tt in range(4):
                ln_transpose(xkv_d, xhatT_kv, tt)
            emit_chunk(qk_units(wk_sb, bk_sb, xhatT_kv, KT, 0, 0))
            for tt in range(4):
                ln_transpose(xq_d, xhatT_q, tt)
            emit_chunk(qk_units(wq_sb, bq_sb, xhatT_q, QT, 0, 0))
            for tt in range(4, TT):
                ln_transpose(xkv_d, xhatT_kv, tt)
            emit_chunk(v_units(0))
            emit_chunk(qk_units(wk_sb, bk_sb, xhatT_kv, KT, 0, 1))
            emit_chunk(v_units(1))
            emit_chunk(v_units(2))
            emit_chunk(v_units(3))
            for tt in range(4, TT):
                ln_transpose(xq_d, xhatT_q, tt)
            emit_chunk(qk_units(wk_sb, bk_sb, xhatT_kv, KT, 1, 0))
            emit_chunk(qk_units(wk_sb, bk_sb, xhatT_kv, KT, 1, 1))
            for tt in range(4, TT):
                emit_chunk(v_units(tt))

            # fillers: remaining projections, ~one matmul per attention step
            # (copybacks pinned to DVE — ACT is saturated by the exp stream)
            fillers = collections.deque()
            fillers.extend(qk_units(wq_sb, bq_sb, xhatT_q, QT, 0, 1, cb_eng="dve"))
            fillers.extend(qk_units(wq_sb, bq_sb, xhatT_q, QT, 1, 0, cb_eng="dve"))
            fillers.extend(qk_units(wq_sb, bq_sb, xhatT_q, QT, 1, 1, cb_eng="dve"))
            for jt in (2, 3):
                fillers.extend(qk_units(wk_sb, bk_sb, xhatT_kv, KT, jt, 0, cb_eng="dve"))
                fillers.extend(qk_units(wk_sb, bk_sb, xhatT_kv, KT, jt, 1, cb_eng="dve"))
                fillers.extend(qk_units(wq_sb, bq_sb, xhatT_q, QT, jt, 0, cb_eng="dve"))
                fillers.extend(qk_units(wq_sb, bq_sb, xhatT_q, QT, jt, 1, cb_eng="dve"))

            late = collections.deque()  # o_proj for token tiles 0-3
            for tt in range(4):
                late.extend(o_units(tt, cb_eng="dve"))

            gstep = 0
            for pair in range(4):
                for ic in range(IC):
                    po = (
                        ps.tile([DH + 1, NQC], F32, tag="po0", bufs=1, name="po0"),
                        ps.tile([DH + 1, NQC], F32, tag="po1", bufs=1, name="po1"),
                    )
                    for kt in range(TT):
                        emit_step(pair, ic, kt, po)
                        thresh = AV_THRESH[kt] if gstep >= 8 else LAG
                        while len(pending_av) > thresh:
                            do_av(*pending_av.popleft())
                        quota = 2 if gstep < 12 else 1
                        for _ in range(quota):
                            if fillers:
                                fillers.popleft()()
                        # o_proj(tt0-3) needs every pair's ic0 normalize;
                        # the last one is emitted by the drain during
                        # (pair3, ic1) step 1 — start late fillers after
                        if pair == 3 and ic == 1 and kt >= 2:
                            for _ in range(3):
                                if late:
                                    late.popleft()()
                        gstep += 1
            while fillers:
                fillers.popleft()()
            while pending_av:
                do_av(*pending_av.popleft())
            while late:
                late.popleft()()

            # tail: output projection for token tiles 4-7
            for tt in range(4, TT):
                emit_chunk(o_units(tt))

    nc.compile()
    return nc


_NC_CACHE = {}


def _get_nc():
    if "nc" not in _NC_CACHE:
        _NC_CACHE["nc"] = _build_nc()
    return _NC_CACHE["nc"]


def _prep_in_maps(query, keyvalue, Wq, Wkv, Wo, gamma, beta):
    query = np.ascontiguousarray(query, dtype=np.float32)
    keyvalue = np.ascontiguousarray(keyvalue, dtype=np.float32)
    Wq = np.asarray(Wq, dtype=np.float32)
    Wkv = np.asarray(Wkv, dtype=np.float32)
    Wo = np.ascontiguousarray(Wo, dtype=np.float32)
    gamma = np.asarray(gamma, dtype=np.float32)
    beta = np.asarray(beta, dtype=np.float32)

    # fold LN affine into the projections: (xhat*g + b) @ W = xhat @ (g[:,None]*W) + b @ W
    wq_eff = np.ascontiguousarray((gamma[:, None] * Wq).astype(np.float16))
    wkv_eff = gamma[:, None] * Wkv
    bq = np.ascontiguousarray(beta @ Wq)
    bkv = beta @ Wkv
    wk_eff = np.ascontiguousarray(wkv_eff[:, :INNER].astype(np.float16))
    wv_eff = np.ascontiguousarray(wkv_eff[:, INNER:].astype(np.float16))
    bk = np.ascontiguousarray(bkv[:INNER])
    bv = np.ascontiguousarray(bkv[INNER:])

    return [
        dict(
            xq=np.ascontiguousarray(query[b]),
            xkv=np.ascontiguousarray(keyvalue[b]),
            wq=wq_eff,
            wk=wk_eff,
            wv=wv_eff,
            wo=Wo,
            bq=bq,
            bk=bk,
            bv=bv,
        )
        for b in range(B)
    ]


def run_sharded(inputs, **spmd_kwargs):
    """Run the SPMD kernel; returns (stacked output [B, N, D], BassKernelResults)."""
    nc = _get_nc()
    in_maps = _prep_in_maps(**inputs)
    r = run_bass_kernel_spmd(nc, in_maps, core_ids=list(range(B)), **spmd_kwargs)
    out = np.stack([r.results[b]["out"] for b in range(B)], axis=0)
    return out, r


def kernel(query, keyvalue, Wq, Wkv, Wo, gamma, beta):
    out, _ = run_sharded(
        dict(query=query, keyvalue=keyvalue, Wq=Wq, Wkv=Wkv, Wo=Wo, gamma=gamma, beta=beta)
    )
    return out

